# revision 9
# baseline (speedup 1.0000x reference)
"""MultiLoraLinear Trainium2 kernel.

Problem: x [8, 2048, 4096] f32, adapter_ids [8] int, weight [16, 64, 4096] f32
         out[b] = x[b] @ weight[adapter_ids[b]].T         -> [8, 2048, 64] f32

Sharding: data-parallel over batch. B == n_cores == 8, so each NeuronCore owns
one batch element. The adapter gather (MoE routing) happens on host: each core
receives only the single [64, 4096] adapter it needs, pre-transposed/tiled.

Per-core compute: out [2048, 64] = x_b [2048, 4096] @ wT [4096, 64].
This is DMA-bound (32 MB of x per core / ~358 GB/s HBM ~= 90 us), so the
kernel keeps the fp32 matmul path (4 cyc/row, measured ~416 ns/MM -> 213 us
PE, PE-bound) OFF the critical path by using an exact bf16 hi/lo split with
fp32 PSUM accumulation:

    x = xh + xl, w = wh + wl (bf16 hi + bf16 residual)
    out ~= wh.x_hi + wl.x_hi + wh.x_lo          (lo.lo term ~2^-18, dropped)

measured rel err vs fp32 reference: 4.4e-06 (bf16 products are exact in the
PE's fp32 accumulate; error comes from the 16-bit effective mantissa of the
hi+lo pair and the dropped lo.lo term).

The PE contracts along the partition dim, so x is host-pre-tiled IN-major:
xhl[kc, p, c, h, s] (kc = K-chunk pair, p = IN%128 partition, c = chunk in
pair, h = hi/lo plane, s = sequence). Each K-chunk-pair is one fully
contiguous 4 MB DMA with 16 KB contiguous per partition line.

Matmuls: stationary = [wh | wl] column-packed [128, 128], moving = x chunk
[128, 512]. One stream of xh produces both wh.xh (PSUM rows 0:64) and wl.xh
(rows 64:128); a second 64-col pass accumulates wh.xl into rows 0:64. The
hi/lo fold is a single DVE add at the end. 256 bf16 MMs ~= 55 us << DMA.

Measured (512-rep hardware-loop wall-clock slope, 8 cores): ~111 us/rep.
Pure-DMA probe of the same traffic: ~98 us. PE-only probe: ~55 us.
"""

import numpy as np
import ml_dtypes

import concourse.bass as bass
import concourse.tile as tile
from concourse import mybir
from concourse import bass_utils

B, S, IN, OUT, L = 8, 2048, 4096, 64, 16
N_CORES = 8
P = 128
KO = IN // P     # 32 contraction chunks of 128
CH = 2           # K-chunks per DMA (4 MB per transfer)
NCH = KO // CH
S4 = S // 512    # moving-dim chunks of 512 (PSUM bank limit)

F32 = mybir.dt.float32
BF16 = mybir.dt.bfloat16


def _split_sync_waits(nc):
    """walrus in this image supports very few sem-wait slots per instruction
    (fp32 Matmult rejects even 2). Move excess waits onto InstEventSemaphore
    carriers inserted immediately before the instruction on the same engine —
    same program point, so ordering semantics are unchanged."""
    counter = [0]

    def _carrier(engine, wait):
        counter[0] += 1
        e = mybir.InstEventSemaphore(name=f"wsplit-{counter[0]}", ins=[], outs=[])
        e.engine = engine
        e.sync_info = mybir.SyncInfo(on_wait=[wait], on_update=[])
        return e

    for f in nc.m.functions:
        for bb in f.blocks:
            new_insts = []
            for inst in bb.instructions:
                si = inst.sync_info
                waits = list(si.on_wait) if si and si.on_wait else []
                cap = 0 if isinstance(inst, mybir.InstMatmult) else 1
                if len(waits) > cap:
                    keep = waits[:cap]
                    for w in waits[cap:]:
                        c = _carrier(inst.engine, w)
                        nc.register_instruction(c, overwrite=True)
                        new_insts.append(c)
                    inst.sync_info = mybir.SyncInfo(
                        on_wait=keep, on_update=list(si.on_update or [])
                    )
                new_insts.append(inst)
            bb.instructions[:] = new_insts


def build_nc(n_rep: int = 1, x_bufs: int = 4):
    """Build the per-core Bass program. n_rep > 1 wraps the computation in a
    hardware For_i loop (same I/O, output overwritten) so harnesses can
    measure steady-state HW time by wall-clock slope; grading uses n_rep=1."""
    nc = bass.Bass("TRN2", target_bir_lowering=False, debug=False)
    x_ap = nc.dram_tensor("xhl", [NCH, P, CH, 2, S], BF16, kind="ExternalInput").ap()
    w_ap = nc.dram_tensor("wt", [P, KO, 2, OUT], BF16, kind="ExternalInput").ap()
    o_ap = nc.dram_tensor("out", [OUT, S], F32, kind="ExternalOutput").ap()

    with tile.TileContext(nc) as tc:
        with (
            tc.tile_pool(name="wpool", bufs=1) as wpool,
            tc.tile_pool(name="xpool", bufs=x_bufs) as xpool,
            tc.tile_pool(name="opool", bufs=2) as opool,
            tc.tile_pool(name="pspool", bufs=1, space="PSUM") as pspool,
        ):
            w_sb = wpool.tile([P, KO, 2, OUT], BF16)
            # SWDGE ring for the 1 MB weight preload so the x stream starts
            # immediately on the HWDGE ring.
            nc.gpsimd.dma_start(w_sb[:], w_ap[:])

            def body():
                pss = [
                    pspool.tile([P, 512], F32, tag=f"ps{s4}", name=f"ps{s4}")
                    for s4 in range(S4)
                ]
                for kc in range(NCH):
                    xt = xpool.tile([P, CH, 2, S], BF16, tag="xhl")
                    nc.sync.dma_start(xt[:], x_ap[kc])
                    for c in range(CH):
                        ko = kc * CH + c
                        w_pk = w_sb[:, ko, :, :]   # [128, 2*OUT] packed [wh|wl]
                        w_hi = w_sb[:, ko, 0, :]   # [128, OUT]
                        for s4 in range(S4):
                            xs_h = xt[:, c, 0, s4 * 512:(s4 + 1) * 512]
                            xs_l = xt[:, c, 1, s4 * 512:(s4 + 1) * 512]
                            nc.tensor.matmul(
                                pss[s4][:, :], w_pk, xs_h,
                                start=(ko == 0), stop=False,
                                skip_group_check=True,
                            )
                            nc.tensor.matmul(
                                pss[s4][:OUT, :], w_hi, xs_l,
                                start=False, stop=(ko == KO - 1),
                                skip_group_check=True,
                            )
                for s4 in range(S4):
                    ot = opool.tile([OUT, 512], F32, tag="ot")
                    nc.scalar.copy(ot[:], pss[s4][OUT:, :])
                    nc.vector.tensor_add(ot[:], ot[:], pss[s4][:OUT, :])
                    nc.sync.dma_start(o_ap[:, s4 * 512:(s4 + 1) * 512], ot[:])

            if n_rep == 1:
                body()
            else:
                with tc.For_i(0, n_rep, 1):
                    body()
    _split_sync_waits(nc)
    return nc


def make_in_maps(x: np.ndarray, adapter_ids: np.ndarray, weight: np.ndarray):
    """Host-side sharding: per-core adapter gather + bf16 hi/lo split + tiling.

    xhl[kc, p, c, h, s] = split(x[b, s, (kc*CH+c)*128 + p])[h]
    wt[p, ko, h, o]     = split(weight[id_b, o, ko*128 + p])[h]
    """
    x = np.asarray(x, dtype=np.float32)
    ids = np.asarray(adapter_ids).astype(np.int64)
    w = np.asarray(weight, dtype=np.float32)
    in_maps = []
    for b in range(B):
        xa = np.ascontiguousarray(x[b].T.reshape(KO, P, S))
        xh = xa.astype(ml_dtypes.bfloat16)
        xl = (xa - xh.astype(np.float32)).astype(ml_dtypes.bfloat16)
        xhl = np.stack([xh, xl], axis=1)                      # [KO, 2, P, S]
        xhl = np.ascontiguousarray(
            xhl.reshape(NCH, CH, 2, P, S).transpose(0, 3, 1, 2, 4)
        )                                                     # [NCH, P, CH, 2, S]
        wsel = w[int(ids[b])]                                 # [OUT, IN]
        wt = np.ascontiguousarray(wsel.T.reshape(KO, P, OUT).transpose(1, 0, 2))
        wh = wt.astype(ml_dtypes.bfloat16)
        wl = (wt - wh.astype(np.float32)).astype(ml_dtypes.bfloat16)
        wpk = np.ascontiguousarray(np.stack([wh, wl], axis=2))  # [P, KO, 2, OUT]
        in_maps.append({"xhl": xhl, "wt": wpk})
    return in_maps


_NC_CACHE = {}


def kernel(x, adapter_ids, weight):
    x = np.asarray(x)
    assert x.shape == (B, S, IN), x.shape
    if "nc" not in _NC_CACHE:
        _NC_CACHE["nc"] = build_nc()
    nc = _NC_CACHE["nc"]
    in_maps = make_in_maps(x, adapter_ids, weight)
    res = bass_utils.run_bass_kernel_spmd(
        nc, in_maps, core_ids=list(range(N_CORES)), trace=False
    )
    out = np.stack(
        [res.results[b]["out"].T for b in range(B)], axis=0
    )
    return np.ascontiguousarray(out, dtype=np.float32)
